# revision 1
# baseline (speedup 1.0000x reference)
"""Trainium2 Bass kernel for nn_CrosslayerDecoder.

Reference computation:
    out[:, l, :] = sum_{i<=l} features[:, i, :] @ W_l[i]  + b[l]
with B=64, L=12, DF=4096, DA=768 (fp32).

Memory-bound on ~981 MB of fp32 weights (each read once).  Eight
specialized 1-core Bass programs run concurrently, one per NeuronCore.
fp32 operands are split on the host into bf16 hi+lo; a 128-column packed
stationary operand ([fh|fl] and [fl|fh]) computes both split products of
each weight stream in a single matmul, so one k-tile costs 4 matmuls of
M=128 instead of 6 of M=64, and all four cross products accumulate
(exact (fh+fl)(wh+wl) split, ~4e-6 rel err).

Global work = 78 pairs x 4 chunks = 312 weight chunks (8 k-tiles each).
Each core gets exactly 39 consecutive chunks (perfect byte balance, no
padding).  A pair whose chunks span a core boundary is split by k-range;
each core emits one partial output per pair-segment and the host sums
segments into layers.  Distinct feature tiles are loaded once per core and
stay resident in SBUF (dedup vs the SPMD version's per-pair reloads).
"""

import numpy as np
import ml_dtypes

import concourse.mybir as mybir
import concourse.tile as tile
from concourse import bacc

B, L, DF, DA = 64, 12, 4096, 768
NCORES = 8
P = 128
KT = DF // P             # 32 k-tiles per pair
KS = 8                   # k-tiles per chunk
CPP = KT // KS           # 4 chunks per pair
NH = DA // 2             # 384

BF16 = ml_dtypes.bfloat16

_PAIRS = [(l, i) for i in range(L) for l in range(i, L)]
assert len(_PAIRS) == 78

# global chunk list: (pair_idx, chunk_in_pair)
_CHUNKS = [(pi, c) for pi in range(len(_PAIRS)) for c in range(CPP)]
assert len(_CHUNKS) == 312 and 312 % NCORES == 0
_PER = 312 // NCORES     # 39 chunks per core


def _core_plan(core):
    """Segments for one core: (l, i, islot, chunk_lo, chunk_hi) per segment.

    chunk range is within the pair (0..CPP); islot indexes this core's
    distinct-feature table.
    """
    chunks = _CHUNKS[core * _PER : (core + 1) * _PER]
    segs = []
    for pi, c in chunks:
        if segs and segs[-1][0] == pi and segs[-1][2] == c:
            segs[-1][2] += 1
        else:
            segs.append([pi, c, c + 1])
    plan = []
    islots = {}
    for pi, c0, c1 in segs:
        l, i = _PAIRS[pi]
        if i not in islots:
            islots[i] = len(islots)
        plan.append((l, i, islots[i], c0, c1))
    return plan, sorted(islots, key=islots.get)


_PLANS = [_core_plan(c) for c in range(NCORES)]
_NC_CACHE = [None] * NCORES


def _build_program(core):
    if _NC_CACHE[core] is not None:
        return _NC_CACHE[core]
    plan, i_list = _PLANS[core]
    n_seg = len(plan)
    n_islot = len(i_list)

    dt = mybir.dt.bfloat16
    nc = bacc.Bacc("TRN2", target_bir_lowering=False, debug=False)
    fh_in = nc.dram_tensor(
        "f_pk", [n_islot, P, KT * 2 * B], dt, kind="ExternalInput"
    ).ap()
    wh_in = nc.dram_tensor("w_hi", [_PER, P, KS * DA], dt, kind="ExternalInput").ap()
    wl_in = nc.dram_tensor("w_lo", [_PER, P, KS * DA], dt, kind="ExternalInput").ap()
    o_out = nc.dram_tensor(
        "out", [n_seg, P, DA], mybir.dt.float32, kind="ExternalOutput"
    ).ap()

    with tile.TileContext(nc) as tc:
        with (
            tc.tile_pool(name="f", bufs=1) as fpool,
            tc.tile_pool(name="w", bufs=4) as wpool,
            tc.tile_pool(name="ps", bufs=2, space="PSUM") as pspool,
            tc.tile_pool(name="o", bufs=2) as opool,
        ):
            # resident packed feature tiles, loaded/built once each.
            # pack1 = [fh_k | fl_k] per k-tile (sent by host); pack2 is the
            # half-swapped copy [fl_k | fh_k], built on-chip with two
            # strided DVE copies.  A 128-col stationary operand computes
            # both bf16-split products of one weight stream in a single
            # matmul (rows 0-63 and 64-127 of PSUM; host folds the halves).
            pk1_t, pk2_t = [], []
            for j in range(n_islot):
                pk1 = fpool.tile([P, KT * 2 * B], dt, tag=f"pk1_{j}")
                (nc.sync if j % 2 == 0 else nc.scalar).dma_start(
                    out=pk1[:], in_=fh_in[j]
                )
                pk2 = fpool.tile([P, KT * 2 * B], dt, tag=f"pk2_{j}")
                p1v = pk1[:].rearrange("p (k m) -> p k m", k=KT)
                p2v = pk2[:].rearrange("p (k m) -> p k m", k=KT)
                nc.vector.tensor_copy(p2v[:, :, 0:B], p1v[:, :, B : 2 * B])
                nc.vector.tensor_copy(p2v[:, :, B : 2 * B], p1v[:, :, 0:B])
                pk1_t.append(pk1)
                pk2_t.append(pk2)

            gchunk = 0  # running index into this core's 39 weight chunks
            for seg_idx, (l, i, islot, c0, c1) in enumerate(plan):
                ps_a = pspool.tile([P, NH], mybir.dt.float32)
                ps_b = pspool.tile([P, NH], mybir.dt.float32)
                nchunks = c1 - c0
                for cc in range(nchunks):
                    wh = wpool.tile([P, KS * DA], dt, tag="wh")
                    wl = wpool.tile([P, KS * DA], dt, tag="wl")
                    ring_a = nc.sync if gchunk % 2 == 0 else nc.scalar
                    ring_b = nc.scalar if gchunk % 2 == 0 else nc.sync
                    ring_a.dma_start(out=wh[:], in_=wh_in[gchunk])
                    ring_b.dma_start(out=wl[:], in_=wl_in[gchunk])
                    for s in range(KS):
                        k = (c0 + cc) * KS + s       # k-tile within the pair
                        l1 = pk1_t[islot][:, k * 2 * B : (k + 1) * 2 * B]
                        l2 = pk2_t[islot][:, k * 2 * B : (k + 1) * 2 * B]
                        whA = wh[:, s * DA : s * DA + NH]
                        whB = wh[:, s * DA + NH : (s + 1) * DA]
                        wlA = wl[:, s * DA : s * DA + NH]
                        wlB = wl[:, s * DA + NH : (s + 1) * DA]
                        first = cc == 0 and s == 0
                        last = cc == nchunks - 1 and s == KS - 1
                        nc.tensor.matmul(ps_a[:], lhsT=l1, rhs=whA, start=first, stop=False)
                        nc.tensor.matmul(ps_b[:], lhsT=l1, rhs=whB, start=first, stop=False)
                        nc.tensor.matmul(ps_a[:], lhsT=l2, rhs=wlA, start=False, stop=last)
                        nc.tensor.matmul(ps_b[:], lhsT=l2, rhs=wlB, start=False, stop=last)
                    gchunk += 1
                ot = opool.tile([P, DA], mybir.dt.float32)
                nc.vector.tensor_copy(ot[:, :NH], ps_a[:])
                nc.vector.tensor_copy(ot[:, NH:], ps_b[:])
                (nc.sync if seg_idx % 2 == 0 else nc.scalar).dma_start(
                    out=o_out[seg_idx], in_=ot[:]
                )
    nc.compile()
    _NC_CACHE[core] = nc
    return nc


def _split_bf16(x32):
    hi = x32.astype(BF16)
    lo = (x32 - hi.astype(np.float32)).astype(BF16)
    return hi, lo


def _prep_inputs(features, Ws):
    features = np.ascontiguousarray(np.asarray(features, dtype=np.float32))
    pk_tiles = {}
    for i in range(L):
        x = features[:, i, :]
        t = np.ascontiguousarray(x.T.reshape(KT, P, B).transpose(1, 0, 2))
        hi, lo = _split_bf16(t)                       # [P, KT, B] each
        pk = np.concatenate([hi, lo], axis=2)         # [P, KT, 2B] = [fh|fl]
        pk_tiles[i] = np.ascontiguousarray(pk.reshape(P, KT * 2 * B))

    # per-pair packed weight chunks [CPP, P, KS*DA] hi/lo, built lazily
    packed = {}

    def pair_chunks(pi):
        if pi not in packed:
            l, i = _PAIRS[pi]
            w32 = np.asarray(Ws[l][i], dtype=np.float32)
            hi, lo = _split_bf16(w32)

            def pack(x):
                return np.ascontiguousarray(
                    x.reshape(CPP, KS, P, DA).transpose(0, 2, 1, 3).reshape(CPP, P, KS * DA)
                )

            packed[pi] = (pack(hi), pack(lo))
        return packed[pi]

    in_maps = []
    for core in range(NCORES):
        plan, i_list = _PLANS[core]
        fpk = np.stack([pk_tiles[i] for i in i_list])
        wh = np.empty((_PER, P, KS * DA), dtype=BF16)
        wl = np.empty((_PER, P, KS * DA), dtype=BF16)
        for j, (pi, c) in enumerate(_CHUNKS[core * _PER : (core + 1) * _PER]):
            ph, pl = pair_chunks(pi)
            wh[j] = ph[c]
            wl[j] = pl[c]
        in_maps.append({"f_pk": fpk, "w_hi": wh, "w_lo": wl})
    return in_maps


def _assemble(results, b):
    out = np.zeros((B, L, DA), dtype=np.float32)
    for core in range(NCORES):
        plan, _ = _PLANS[core]
        o = np.asarray(results[core]["out"], dtype=np.float32)
        for seg_idx, (l, _i, _islot, _c0, _c1) in enumerate(plan):
            out[:, l, :] += o[seg_idx, :B] + o[seg_idx, B:]
    out += np.asarray(b, dtype=np.float32)[None, :, :]
    return out


def _run_all(in_maps):
    """Dispatch the 8 per-core programs concurrently (thread per core)."""
    import concurrent.futures as cf

    import jax

    from concourse import bass2jax

    devices = jax.devices()[:NCORES]
    ncs = [_build_program(c) for c in range(NCORES)]

    def one(c):
        with jax.default_device(devices[c]):
            return bass2jax.run_bass_via_pjrt(ncs[c], [in_maps[c]], n_cores=1)[0]

    with cf.ThreadPoolExecutor(max_workers=NCORES) as ex:
        results = list(ex.map(one, range(NCORES)))
    return results


def _run_all_retry(in_maps, attempts=3):
    last = None
    for a in range(attempts):
        try:
            return _run_all(in_maps)
        except Exception as e:  # transient NRT_EXEC_UNIT_UNRECOVERABLE seen
            last = e
            print(f"kernel run attempt {a} failed ({e}); retrying")
    raise last


def run(inputs: dict, trace: bool = False, tmpdir: str | None = None):
    Ws = [np.asarray(inputs[f"W_{l}"], dtype=np.float32) for l in range(L)]
    in_maps = _prep_inputs(inputs["features"], Ws)

    if not trace:
        results = _run_all_retry(in_maps)
        return _assemble(results, inputs["b"]), None

    # tracing: wrap execution with the axon NTFF hook, then convert each
    # captured NTFF (one per core executable) to json via neuron-profile.
    import glob
    import json
    import re
    import subprocess
    import tempfile
    from dataclasses import dataclass

    from antenv.axon_hooks import get_axon_ntff_profile_hook

    hook = get_axon_ntff_profile_hook()
    neff_dir = tmpdir or tempfile.mkdtemp()
    with hook(neff_dir, [0]):
        results = _run_all(in_maps)
    out = _assemble(results, inputs["b"])

    times = []
    for ntff in sorted(glob.glob(neff_dir + "/*_body*.ntff")):
        m = re.search(r"(executable\d+)", ntff)
        neffs = glob.glob(neff_dir + f"/*{m.group(1)}.neff") if m else []
        if not neffs:
            continue
        jf = ntff + ".json"
        try:
            subprocess.check_call(
                [
                    "neuron-profile", "view", "--ignore-nc-buf-usage",
                    "-s", ntff, "-n", neffs[0],
                    "--output-format=json", f"--output-file={jf}",
                ],
                stdout=subprocess.DEVNULL, stderr=subprocess.DEVNULL,
            )
            with open(jf) as f:
                summ = json.load(f)["summary"][0]
            times.append((summ["total_time"] * 1e9, summ.get("nc_idx"), jf))
        except Exception as e:
            print("ntff convert failed:", ntff, e)
    times.sort(reverse=True)
    for t, nc_idx, jf in times:
        print(f"  core nc_idx={nc_idx}: {t:.0f} ns  ({jf})")

    @dataclass
    class R:
        exec_time_ns: int | None
        mean_exec_time_ns: float | None
        instructions_and_trace = None
        profile_json = None

    res = R(
        exec_time_ns=int(times[0][0]) if times else None,
        mean_exec_time_ns=(sum(t for t, _, _ in times) / len(times)) if times else None,
    )
    return out, res


def kernel(**inputs) -> np.ndarray:
    out, _ = run(inputs)
    return out

